# revision 2
# baseline (speedup 1.0000x reference)
"""Trainium2 Bass kernel for the EnhancedBCMLayer (block-circulant matrix layer).

Math: out[B, 16f+i] = sum_{g,j} iv[f,g,(i-j)%16] * x[B,16g+j] + b[16f+i]
i.e. per (f,g) 16x16 block the weight is circulant. Computed in the rfft
domain: for each of the 9 rfft bins k, Yhat_k[B,f] = sum_g Phat_k[f,g] *
Xhat_k[B,g] (complex). The cheap length-16 rfft/irfft transforms run on the
host; the expensive einsum over g runs on 8 NeuronCores (data-parallel over
the batch), packed as 32 fp32 matmuls of [128,128] @ [128,512]:

  - complex bins pair (Re,Im) components; contraction K = (2 comps x 64 g),
    output M = (2 comps x 64 f), with the 2x2 complex-multiply block structure
    baked into the host-built stationary weights.
  - the two real bins (0 and 8) share one pair slot with a block-diagonal
    weight.
"""

import numpy as np

import concourse.mybir as mybir
import concourse.tile as tile
from concourse import bacc
from concourse.bass_utils import run_bass_kernel_spmd

N_CORES = 8
BATCH = 4096
IN_FEATURES = 2048
OUT_FEATURES = 2048
BS = 16          # circulant block size
NB = 128         # feature blocks (f and g)
BINS = 9         # rfft bins of length-16 signal
NPAIR = 8        # component pairs: (re0,re8), (re1,im1), ..., (re7,im7)
BC = BATCH // N_CORES  # 512 batch rows per core

_CACHED = {}


def _emit_body(nc, tc, pools, xin, win, yout):
    f32 = mybir.dt.float32
    wp, xp, op, ps = pools
    # Load everything up front; Tile overlaps DMA with compute.
    xtiles = {}
    wtiles = {}
    for p in range(NPAIR):
        for gh in range(2):
            t = xp.tile([128, BC], f32, tag=f"x{p}_{gh}")
            nc.sync.dma_start(t[:], xin[p, gh])
            xtiles[p, gh] = t
        for fh in range(2):
            for gh in range(2):
                w = wp.tile([128, 128], f32, tag=f"w{p}_{fh}_{gh}")
                nc.sync.dma_start(w[:], win[p, fh, gh])
                wtiles[p, fh, gh] = w
    for p in range(NPAIR):
        for fh in range(2):
            acc = ps.tile([128, BC], f32, tag="acc")
            nc.tensor.matmul(acc[:], wtiles[p, fh, 0][:], xtiles[p, 0][:],
                             start=True, stop=False)
            nc.tensor.matmul(acc[:], wtiles[p, fh, 1][:], xtiles[p, 1][:],
                             start=False, stop=True)
            o = op.tile([128, BC], f32, tag="o")
            nc.vector.tensor_copy(out=o[:], in_=acc[:])
            nc.sync.dma_start(yout[p, fh], o[:])


def _build_nc(loop_reps=0):
    """Build the Bass program (one NEFF, SPMD across 8 cores).

    loop_reps > 0 wraps the body in a For_i loop running it that many times
    (benchmarking variant; output identical since iterations are idempotent).
    """
    nc = bacc.Bacc("TRN2", target_bir_lowering=False, num_devices=N_CORES)
    f32 = mybir.dt.float32
    xin = nc.dram_tensor("xin", [NPAIR, 2, 128, BC], f32, kind="ExternalInput")
    win = nc.dram_tensor("win", [NPAIR, 2, 2, 128, 128], f32, kind="ExternalInput")
    yout = nc.dram_tensor("yout", [NPAIR, 2, 128, BC], f32, kind="ExternalOutput")

    with tile.TileContext(nc) as tc:
        with (
            tc.tile_pool(name="wp", bufs=1) as wp,
            tc.tile_pool(name="xp", bufs=1) as xp,
            tc.tile_pool(name="op", bufs=4) as op,
            tc.tile_pool(name="ps", bufs=4, space="PSUM") as ps,
        ):
            pools = (wp, xp, op, ps)
            if loop_reps:
                with tc.For_i(0, loop_reps, 1):
                    _emit_body(nc, tc, pools, xin, win, yout)
            else:
                _emit_body(nc, tc, pools, xin, win, yout)
    nc.compile()
    return nc


def _host_prep_weights(index_vectors):
    """Host: rfft the circulant generators and pack the 32 stationary 128x128
    lhsT weight tiles win[pair, fh, gh, K=(cin*64+g'), M=(cout*64+f')]."""
    Phat = np.fft.rfft(index_vectors.astype(np.float64), axis=-1)  # (f,g,9)
    win = np.zeros((NPAIR, 2, 2, 128, 128), dtype=np.float64)
    for p in range(NPAIR):
        for fh in range(2):
            for gh in range(2):
                fs = slice(64 * fh, 64 * fh + 64)
                gs = slice(64 * gh, 64 * gh + 64)
                if p == 0:
                    b0 = Phat[fs, gs, 0].real.T  # [g', f']
                    b8 = Phat[fs, gs, 8].real.T
                    win[p, fh, gh, 0:64, 0:64] = b0
                    win[p, fh, gh, 64:128, 64:128] = b8
                else:
                    pr = Phat[fs, gs, p].real.T
                    pi = Phat[fs, gs, p].imag.T
                    win[p, fh, gh, 0:64, 0:64] = pr      # Xr -> Yr
                    win[p, fh, gh, 64:128, 0:64] = -pi   # Xi -> Yr
                    win[p, fh, gh, 0:64, 64:128] = pi    # Xr -> Yi
                    win[p, fh, gh, 64:128, 64:128] = pr  # Xi -> Yi
    return np.ascontiguousarray(win, dtype=np.float32)


def _host_prep_x(x):
    """Host: rfft the input blocks and lay out per-core rhs tiles
    xin[pair, gh, K=(comp*64+g'), b]."""
    Xf = np.fft.rfft(x.reshape(BATCH, NB, BS), axis=-1)  # (B, g, 9) complex128
    # comps[pair, comp] as (B, g) slabs
    xin = np.empty((N_CORES, NPAIR, 2, 2, 64, BC), dtype=np.float32)
    XfT = Xf.transpose(1, 2, 0)  # (g, bin, B)
    for p in range(NPAIR):
        if p == 0:
            c0 = XfT[:, 0].real
            c1 = XfT[:, 8].real
        else:
            c0 = XfT[:, p].real
            c1 = XfT[:, p].imag
        for gh in range(2):
            gs = slice(64 * gh, 64 * gh + 64)
            for core in range(N_CORES):
                bsl = slice(core * BC, (core + 1) * BC)
                xin[core, p, gh, 0] = c0[gs, bsl]
                xin[core, p, gh, 1] = c1[gs, bsl]
    return xin.reshape(N_CORES, NPAIR, 2, 128, BC)


def _host_post(youts, b):
    """Host: reassemble Yhat bins from the 8 cores' outputs, irfft, add bias."""
    Yf = np.empty((BATCH, NB, BINS), dtype=np.complex128)
    for core in range(N_CORES):
        y = youts[core]  # (NPAIR, 2, 128, BC)
        bsl = slice(core * BC, (core + 1) * BC)
        # y[p, fh, cout*64+f', b]
        yr = np.concatenate([y[:, 0, 0:64], y[:, 1, 0:64]], axis=1)    # (NPAIR,128f,BC)
        yi = np.concatenate([y[:, 0, 64:128], y[:, 1, 64:128]], axis=1)
        yrT = yr.transpose(2, 1, 0)  # (BC, f, NPAIR)
        yiT = yi.transpose(2, 1, 0)
        Yf[bsl, :, 0] = yrT[:, :, 0]
        Yf[bsl, :, 8] = yiT[:, :, 0]
        Yf[bsl, :, 1:8] = yrT[:, :, 1:] + 1j * yiT[:, :, 1:]
    out = np.fft.irfft(Yf, n=BS, axis=-1).reshape(BATCH, OUT_FEATURES)
    return (out + b.astype(np.float64)).astype(np.float32)


def run(x, index_vectors, b, trace=False):
    if "nc" not in _CACHED:
        _CACHED["nc"] = _build_nc()
    nc = _CACHED["nc"]
    win = _host_prep_weights(np.asarray(index_vectors))
    xin = _host_prep_x(np.asarray(x))
    in_maps = [{"xin": xin[c], "win": win} for c in range(N_CORES)]
    res = run_bass_kernel_spmd(nc, in_maps, core_ids=list(range(N_CORES)),
                               trace=trace)
    youts = [res.results[c]["yout"] for c in range(N_CORES)]
    out = _host_post(youts, np.asarray(b))
    return out, res


def kernel(x, index_vectors, b):
    out, _ = run(x, index_vectors, b)
    return out


# revision 6
# speedup vs baseline: 1.5705x; 1.5705x over previous
"""Trainium2 Bass kernel for the EnhancedBCMLayer (block-circulant matrix layer).

Math: out[B, 16f+i] = sum_{g,j} iv[f,g,(i-j)%16] * x[B,16g+j] + b[16f+i]
i.e. per (f,g) 16x16 block the weight is circulant. Computed in the rfft
domain: for each of the 9 rfft bins k, Yhat_k[B,f] = sum_g Phat_k[f,g] *
Xhat_k[B,g] (complex). The cheap length-16 rfft/irfft transforms run on the
host; the expensive einsum over g runs on 8 NeuronCores (data-parallel over
the batch), packed as 32 fp32 matmuls of [128,128] @ [128,512]:

  - complex bins pair (Re,Im) components; contraction K = (2 comps x 64 g),
    output M = (2 comps x 64 f), with the 2x2 complex-multiply block structure
    baked into the host-built stationary weights.
  - the two real bins (0 and 8) share one pair slot with a block-diagonal
    weight.
"""

import numpy as np

import concourse.mybir as mybir
import concourse.tile as tile
from concourse import bacc
from concourse.bass_utils import run_bass_kernel_spmd

N_CORES = 8
BATCH = 4096
IN_FEATURES = 2048
OUT_FEATURES = 2048
BS = 16          # circulant block size
NB = 128         # feature blocks (f and g)
BINS = 9         # rfft bins of length-16 signal
NPAIR = 8        # component pairs: (re0,re8), (re1,im1), ..., (re7,im7)
BC = BATCH // N_CORES  # 512 batch rows per core

_CACHED = {}


def _emit_body(nc, tc, pools, xin, win, yout):
    f32 = mybir.dt.float32
    f32r = mybir.dt.float32r
    wp, xp, op, ps = pools
    # One batched DMA per pair per tensor (the per-DMA sequencer issue cost
    # is ~0.9us, so DMA count matters more than shape here).
    xtiles = {}
    wtiles = {}
    for p in range(NPAIR):
        xt = xp.tile([128, 2, BC], f32r, tag=f"x{p}")
        nc.sync.dma_start(xt[:], xin[p].rearrange("gh k b -> k gh b"))
        xtiles[p] = xt
        wt = wp.tile([128, 4, 128], f32r, tag=f"w{p}")
        nc.sync.dma_start(wt[:], win[p].rearrange("fh gh k m -> k (fh gh) m"))
        wtiles[p] = wt
    for p in range(NPAIR):
        o = op.tile([128, 2, BC], f32, tag="o")
        for fh in range(2):
            acc = ps.tile([128, BC], f32, tag="acc")
            for gh in range(2):
                nc.tensor.matmul(acc[:],
                                 wtiles[p][:, fh * 2 + gh],
                                 xtiles[p][:, gh],
                                 start=(gh == 0), stop=(gh == 1))
            nc.vector.tensor_copy(out=o[:, fh], in_=acc[:])
        nc.sync.dma_start(yout[p].rearrange("fh k b -> k fh b"), o[:])


def _build_nc(loop_reps=0):
    """Build the Bass program (one NEFF, SPMD across 8 cores).

    loop_reps > 0 wraps the body in a For_i loop running it that many times
    (benchmarking variant; output identical since iterations are idempotent).
    """
    nc = bacc.Bacc("TRN2", target_bir_lowering=False, num_devices=N_CORES)
    f32 = mybir.dt.float32
    f32r = mybir.dt.float32r
    xin = nc.dram_tensor("xin", [NPAIR, 2, 128, BC], f32r, kind="ExternalInput")
    win = nc.dram_tensor("win", [NPAIR, 2, 2, 128, 128], f32r, kind="ExternalInput")
    yout = nc.dram_tensor("yout", [NPAIR, 2, 128, BC], f32, kind="ExternalOutput")

    with tile.TileContext(nc) as tc:
        with (
            tc.tile_pool(name="wp", bufs=1) as wp,
            tc.tile_pool(name="xp", bufs=1) as xp,
            tc.tile_pool(name="op", bufs=4) as op,
            tc.tile_pool(name="ps", bufs=4, space="PSUM") as ps,
        ):
            pools = (wp, xp, op, ps)
            if loop_reps:
                with tc.For_i(0, loop_reps, 1):
                    _emit_body(nc, tc, pools, xin, win, yout)
            else:
                _emit_body(nc, tc, pools, xin, win, yout)
    nc.compile()
    return nc


def _host_prep_weights(index_vectors):
    """Host: rfft the circulant generators and pack the 32 stationary 128x128
    lhsT weight tiles win[pair, fh, gh, K=(cin*64+g'), M=(cout*64+f')]."""
    Phat = np.fft.rfft(index_vectors.astype(np.float64), axis=-1)  # (f,g,9)
    win = np.zeros((NPAIR, 2, 2, 128, 128), dtype=np.float64)
    for p in range(NPAIR):
        for fh in range(2):
            for gh in range(2):
                fs = slice(64 * fh, 64 * fh + 64)
                gs = slice(64 * gh, 64 * gh + 64)
                if p == 0:
                    b0 = Phat[fs, gs, 0].real.T  # [g', f']
                    b8 = Phat[fs, gs, 8].real.T
                    win[p, fh, gh, 0:64, 0:64] = b0
                    win[p, fh, gh, 64:128, 64:128] = b8
                else:
                    pr = Phat[fs, gs, p].real.T
                    pi = Phat[fs, gs, p].imag.T
                    win[p, fh, gh, 0:64, 0:64] = pr      # Xr -> Yr
                    win[p, fh, gh, 64:128, 0:64] = -pi   # Xi -> Yr
                    win[p, fh, gh, 0:64, 64:128] = pi    # Xr -> Yi
                    win[p, fh, gh, 64:128, 64:128] = pr  # Xi -> Yi
    return np.ascontiguousarray(win, dtype=np.float32)


def _host_prep_x(x):
    """Host: rfft the input blocks and lay out per-core rhs tiles
    xin[pair, gh, K=(comp*64+g'), b]."""
    Xf = np.fft.rfft(x.reshape(BATCH, NB, BS), axis=-1)  # (B, g, 9) complex128
    # comps[pair, comp] as (B, g) slabs
    xin = np.empty((N_CORES, NPAIR, 2, 2, 64, BC), dtype=np.float32)
    XfT = Xf.transpose(1, 2, 0)  # (g, bin, B)
    for p in range(NPAIR):
        if p == 0:
            c0 = XfT[:, 0].real
            c1 = XfT[:, 8].real
        else:
            c0 = XfT[:, p].real
            c1 = XfT[:, p].imag
        for gh in range(2):
            gs = slice(64 * gh, 64 * gh + 64)
            for core in range(N_CORES):
                bsl = slice(core * BC, (core + 1) * BC)
                xin[core, p, gh, 0] = c0[gs, bsl]
                xin[core, p, gh, 1] = c1[gs, bsl]
    return xin.reshape(N_CORES, NPAIR, 2, 128, BC)


def _host_post(youts, b):
    """Host: reassemble Yhat bins from the 8 cores' outputs, irfft, add bias."""
    Yf = np.empty((BATCH, NB, BINS), dtype=np.complex128)
    for core in range(N_CORES):
        y = youts[core]  # (NPAIR, 2, 128, BC)
        bsl = slice(core * BC, (core + 1) * BC)
        # y[p, fh, cout*64+f', b]
        yr = np.concatenate([y[:, 0, 0:64], y[:, 1, 0:64]], axis=1)    # (NPAIR,128f,BC)
        yi = np.concatenate([y[:, 0, 64:128], y[:, 1, 64:128]], axis=1)
        yrT = yr.transpose(2, 1, 0)  # (BC, f, NPAIR)
        yiT = yi.transpose(2, 1, 0)
        Yf[bsl, :, 0] = yrT[:, :, 0]
        Yf[bsl, :, 8] = yiT[:, :, 0]
        Yf[bsl, :, 1:8] = yrT[:, :, 1:] + 1j * yiT[:, :, 1:]
    out = np.fft.irfft(Yf, n=BS, axis=-1).reshape(BATCH, OUT_FEATURES)
    return (out + b.astype(np.float64)).astype(np.float32)


def run(x, index_vectors, b, trace=False):
    if "nc" not in _CACHED:
        _CACHED["nc"] = _build_nc()
    nc = _CACHED["nc"]
    win = _host_prep_weights(np.asarray(index_vectors))
    xin = _host_prep_x(np.asarray(x))
    in_maps = [{"xin": xin[c], "win": win} for c in range(N_CORES)]
    res = run_bass_kernel_spmd(nc, in_maps, core_ids=list(range(N_CORES)),
                               trace=trace)
    youts = [res.results[c]["yout"] for c in range(N_CORES)]
    out = _host_post(youts, np.asarray(b))
    return out, res


def kernel(x, index_vectors, b):
    out, _ = run(x, index_vectors, b)
    return out
